# revision 30
# baseline (speedup 1.0000x reference)
"""Trainium2 Bass kernel for an 8-batch image-conditioned decoder layer.

Strategy: pure data-parallel over the batch — core c computes batch element c
end-to-end (causal self-attention, cross-attention over the image tokens, both
layernorms, vocab projection).  No collectives.

Schedule v2: the kernel is Tensor-engine bound (~509us of bf16 PE work at
78.6 TF/s), so the layout is organized to keep the PE gapless:
 - the first projection (Q1) runs k-outer across all 8 PSUM banks so it can
   start as soon as the first 384KB k-slab of x/W lands (vs 1.5MB before);
 - host folds biases (bv1 into the residual stream, bv2 into LN1's shift,
   g2/b2 into the vocab weights+bias), removing rank-1 bias matmuls and two
   vector ops per LN2 row;
 - elementwise work is spread (Act: exp/LN-act + Q bias, Vector: reductions +
   K bias + residual, GpSimd: V/P copies) so no chain serializes the PE;
 - one vocab chunk + the 256-wide vocab tail are woven into the
   cross-attention window as PE filler while LN2/x2-transposes drain;
 - the wp stream reuses the attention-weight SBUF ring (slots recycle as the
   attention weights die), and the final vocab group is a single small chunk
   so the last write drains in ~1us.
"""

import os
import sys

for _p in ("/opt/trn_rl_repo", "/root/.axon_site/_ro/trn_rl_repo"):
    if os.path.isdir(_p) and _p not in sys.path:
        sys.path.append(_p)

import numpy as np
import ml_dtypes

BF16 = ml_dtypes.bfloat16

# Problem dims (hardcoded per spec)
V, D, DI, S, B, NI = 32000, 1024, 768, 512, 8, 197
EPS = 1e-5
P = 128
ST = S // P          # 4 seq tiles
DT = D // P          # 8 model-dim tiles
DIT = DI // P        # 6 image-dim tiles
NIT = 2              # image tokens: 197 -> 2 partition tiles (128 + 69)
NI_PAD = 256
CN = 512             # vocab chunk width
NFULL = V // CN      # 62 full chunks
CTAIL = V - NFULL * CN   # 256 tail columns
GRP = 4              # full chunks per steady-state output group
N_CORES = 8
HD = D // 2
SCALE = 1.0 / float(np.sqrt(np.float32(D)))

_CACHE = {}
LAST_RESULTS = None


def _build_program():
    import concourse.bacc as bacc
    import concourse.bass as bass
    import concourse.mybir as mybir
    from concourse.masks import make_identity
    from concourse.tile import TileContext

    f32 = mybir.dt.float32
    bf16 = mybir.dt.bfloat16
    X = mybir.AxisListType.X
    ALU = mybir.AluOpType
    ACT_F = mybir.ActivationFunctionType

    nc = bacc.Bacc("TRN2", target_bir_lowering=False, debug=False,
                   num_devices=N_CORES)

    # ---- I/O ----
    h_x0b = nc.dram_tensor("x0b", [P, ST, D], bf16, kind="ExternalInput")
    h_x0t = nc.dram_tensor("x0t", [P, DT, S], bf16, kind="ExternalInput")
    h_img = nc.dram_tensor("img_t", [P, DIT, NI], bf16, kind="ExternalInput")
    h_wq1 = nc.dram_tensor("wq1", [P, DT, D], bf16, kind="ExternalInput")
    h_wk1 = nc.dram_tensor("wk1", [P, DT, D], bf16, kind="ExternalInput")
    h_wv1 = nc.dram_tensor("wv1", [P, DT, D], bf16, kind="ExternalInput")
    h_wq2 = nc.dram_tensor("wq2", [P, DT, D], bf16, kind="ExternalInput")
    h_wk2 = nc.dram_tensor("wk2", [P, DIT, D], bf16, kind="ExternalInput")
    h_wv2 = nc.dram_tensor("wv2", [P, DIT, D], bf16, kind="ExternalInput")
    h_wp = nc.dram_tensor("wp", [NFULL, P, DT, CN], bf16, kind="ExternalInput")
    h_wpt = nc.dram_tensor("wpt", [P, DT, CTAIL], bf16, kind="ExternalInput")
    h_bqs = nc.dram_tensor("bqs", [P, 4, DT], f32, kind="ExternalInput")
    h_bp = nc.dram_tensor("bp", [V], bf16, kind="ExternalInput")
    h_g1 = nc.dram_tensor("g1", [D], bf16, kind="ExternalInput")
    h_b1 = nc.dram_tensor("b1", [D], bf16, kind="ExternalInput")
    h_out = nc.dram_tensor("out", [S, V], bf16, kind="ExternalOutput")

    def bcast(handle, n, offset=0):
        ap = handle[:]
        return bass.AP(tensor=ap.tensor, offset=offset, ap=[[0, P], [1, n]])

    with TileContext(nc) as tc:
        import contextlib
        ctx = contextlib.ExitStack()
        with ctx:
            const = ctx.enter_context(tc.tile_pool(name="const", bufs=1))
            xs_p = ctx.enter_context(tc.tile_pool(name="xs", bufs=1))
            wqs_p = ctx.enter_context(tc.tile_pool(name="wqs", bufs=2))
            wp_p = ctx.enter_context(tc.tile_pool(name="wpp", bufs=4))
            xb_p = ctx.enter_context(tc.tile_pool(name="xb", bufs=2))
            qk_p = ctx.enter_context(tc.tile_pool(name="qk", bufs=2))
            v_p = ctx.enter_context(tc.tile_pool(name="vp", bufs=2))
            k2t_p = ctx.enter_context(tc.tile_pool(name="k2t", bufs=1))
            pb_p = ctx.enter_context(tc.tile_pool(name="pb", bufs=4))
            pt_p = ctx.enter_context(tc.tile_pool(name="pt", bufs=1))
            dgl_p = ctx.enter_context(tc.tile_pool(name="dgl", bufs=1))
            xpre_p = ctx.enter_context(tc.tile_pool(name="xpre", bufs=2))
            stat_p = ctx.enter_context(tc.tile_pool(name="stat", bufs=4))
            wts_p = ctx.enter_context(tc.tile_pool(name="wts", bufs=4))
            xt_p = ctx.enter_context(tc.tile_pool(name="xt", bufs=1))
            bp_p = ctx.enter_context(tc.tile_pool(name="bpp", bufs=1))
            osb_p = ctx.enter_context(tc.tile_pool(name="osb", bufs=4))
            ps = ctx.enter_context(tc.tile_pool(name="ps", bufs=8, space="PSUM"))

            # ---- startup DMA ----
            # DMA throughput scales with per-partition contiguous run length
            # (1KB rows ~60GB/s, 4KB ~180, 8KB ~280), so the first operands
            # ship as 4KB-row halves/quarters split across two queues.
            x0t_lo = xs_p.tile([P, 4, S], bf16, tag="xtl", name="x0tl")
            nc.sync.dma_start(out=x0t_lo, in_=h_x0t[:, 0:4, :])
            wq1q = []
            for qi, eng in enumerate((nc.scalar, nc.sync, nc.scalar,
                                      nc.scalar)):
                t = wqs_p.tile([P, 2, D], bf16, tag="wq1q", name=f"wq1q{qi}")
                eng.dma_start(out=t, in_=h_wq1[:, 2 * qi:2 * qi + 2, :])
                wq1q.append(t)
            x0t_hi = xs_p.tile([P, 4, S], bf16, tag="xth", name="x0th")
            nc.sync.dma_start(out=x0t_hi, in_=h_x0t[:, 4:DT, :])

            def x0t_of(k):
                return (x0t_lo, k) if k < 4 else (x0t_hi, k - 4)

            # sync: wk1a, then the first wp chunks (dedicated 4-deep ring so
            # the vocab stream builds a lead independent of attention timing)
            wk1a = wts_p.tile([P, DT, HD], bf16, tag="wts", name="wk1a")
            nc.sync.dma_start(out=wk1a, in_=h_wk1[:, :, 0:HD])
            wp_tiles = {}
            wp_emitted = 0

            def emit_wp(upto):
                nonlocal wp_emitted
                while wp_emitted < min(upto, NFULL):
                    c = wp_emitted
                    t = wp_p.tile([P, DT, CN], bf16, tag="wp", name=f"wp{c}")
                    nc.sync.dma_start(out=t, in_=h_wp[c])
                    wp_tiles[c] = t
                    wp_emitted += 1

            emit_wp(4)

            # scalar: small consts + K weight half + img + x0b
            bqall = const.tile([P, 4, DT], f32)
            nc.scalar.dma_start(out=bqall, in_=h_bqs[:])
            wk1b = wts_p.tile([P, DT, HD], bf16, tag="wts", name="wk1b")
            nc.scalar.dma_start(out=wk1b, in_=h_wk1[:, :, HD:D])
            img_sb = const.tile([P, DIT, NI], bf16)
            nc.scalar.dma_start(out=img_sb, in_=h_img[:])
            g1b = const.tile([P, D], bf16)
            nc.scalar.dma_start(out=g1b, in_=bcast(h_g1, D))
            b1b = const.tile([P, D], bf16)
            nc.scalar.dma_start(out=b1b, in_=bcast(h_b1, D))
            x0b = xb_p.tile([P, ST, D], bf16, tag="xb", name="x0b")
            nc.scalar.dma_start(out=x0b, in_=h_x0b[:])

            # gpsimd: V weights first (vproj needs them at ~30us), then the
            # cross-attn weights; all fire early so the software-DGE has
            # nothing to drain at kernel end.
            wv1a = wts_p.tile([P, DT, HD], bf16, tag="wts", name="wv1a")
            nc.gpsimd.dma_start(out=wv1a, in_=h_wv1[:, :, 0:HD])
            wv1b = wts_p.tile([P, DT, HD], bf16, tag="wts", name="wv1b")
            nc.gpsimd.dma_start(out=wv1b, in_=h_wv1[:, :, HD:D])
            wpt_sb = const.tile([P, DT, CTAIL], bf16)
            nc.gpsimd.dma_start(out=wpt_sb, in_=h_wpt[:])
            bpt_bc = const.tile([P, CTAIL], bf16)
            nc.gpsimd.dma_start(out=bpt_bc, in_=bcast(h_bp, CTAIL,
                                                      offset=NFULL * CN))
            bp0_bc = const.tile([P, CN], bf16)
            nc.gpsimd.dma_start(out=bp0_bc, in_=bcast(h_bp, CN))

            # constants (gpsimd/vector compute, after its early triggers)
            ident = const.tile([P, P], bf16)
            make_identity(nc, ident)
            trimask = const.tile([P, P], f32)
            nc.gpsimd.memset(trimask, 0.0)
            nc.gpsimd.affine_select(
                out=trimask, in_=trimask, compare_op=ALU.is_ge, fill=-1e10,
                base=0, pattern=[[-1, P]], channel_multiplier=1)
            epst = const.tile([P, 1], f32)
            nc.vector.memset(epst, EPS)
            V2t = v_p.tile([P, NIT, D], bf16, tag="v")
            nc.vector.memset(V2t[:, 1, :], 0.0)

            # these ride gpsimd after the consts; ring slots free by the time
            # each trigger reaches the head of the queue
            wk2a = wts_p.tile([P, DIT, HD], bf16, tag="wts", name="wk2a")
            nc.gpsimd.dma_start(out=wk2a, in_=h_wk2[:, :, 0:HD])
            wk2b = wts_p.tile([P, DIT, HD], bf16, tag="wts", name="wk2b")
            nc.gpsimd.dma_start(out=wk2b, in_=h_wk2[:, :, HD:D])
            wv2a = wts_p.tile([P, DIT, HD], bf16, tag="wts", name="wv2a")
            nc.gpsimd.dma_start(out=wv2a, in_=h_wv2[:, :, 0:HD])
            wv2b = wts_p.tile([P, DIT, HD], bf16, tag="wts", name="wv2b")
            nc.gpsimd.dma_start(out=wv2b, in_=h_wv2[:, :, HD:D])
            wq2a = wts_p.tile([P, DT, HD], bf16, tag="wts", name="wq2a")
            nc.gpsimd.dma_start(out=wq2a, in_=h_wq2[:, :, 0:HD])
            wq2b = wts_p.tile([P, DT, HD], bf16, tag="wts", name="wq2b")
            nc.gpsimd.dma_start(out=wq2b, in_=h_wq2[:, :, HD:D])

            # ---- Q1 projection, k-outer across all 8 PSUM banks ----
            QT = qk_p.tile([P, DT, S], bf16, tag="qk", name="qt")
            psQ = [ps.tile([P, 512], f32, tag="ps", name=f"psq{m}")
                   for m in range(DT)]
            for k in range(DT):
                xt, kk = x0t_of(k)
                wq, kq = wq1q[k // 2], k % 2
                for m in range(DT):
                    nc.tensor.matmul(psQ[m],
                                     lhsT=wq[:, kq, m * P:(m + 1) * P],
                                     rhs=xt[:, kk, :],
                                     start=(k == 0), stop=(k == DT - 1))
            for m in range(DT):
                nc.scalar.activation(out=QT[:, m, :], in_=psQ[m],
                                     func=ACT_F.Identity,
                                     bias=bqall[:, 0, m:m + 1], scale=1.0)

            # ---- K1 projection, m-outer (x0t fully resident by now) ----
            # note: no bk1 — a bias on K shifts each score row by a constant,
            # which softmax cancels exactly.
            KT = qk_p.tile([P, DT, S], bf16, tag="qk", name="kt")
            for m in range(DT):
                w_sb, mb = (wk1a, m * P) if m < 4 else (wk1b, (m - 4) * P)
                pm = ps.tile([P, 512], f32, tag="ps", name="pmk")
                for k in range(DT):
                    xt, kk = x0t_of(k)
                    nc.tensor.matmul(pm, lhsT=w_sb[:, k, mb:mb + P],
                                     rhs=xt[:, kk, :],
                                     start=(k == 0), stop=(k == DT - 1))
                nc.scalar.copy(out=KT[:, m, :], in_=pm)

            # ---- V projection first: its matmuls cover the Act window that
            # produces the Q/K copies scores depend on ----
            Vt = v_p.tile([P, ST, D], bf16, tag="v")
            for a in range(ST):
                for nh in range(2):
                    wv = wv1a if nh == 0 else wv1b
                    pm = ps.tile([P, 512], f32, tag="ps")
                    for k in range(DT):
                        xt, kk = x0t_of(k)
                        nc.tensor.matmul(
                            pm, lhsT=xt[:, kk, a * P:(a + 1) * P],
                            rhs=wv[:, k, :],
                            start=(k == 0), stop=(k == DT - 1))
                    nc.vector.tensor_scalar_add(
                        Vt[:, a, nh * 512:(nh + 1) * 512], pm, 0.0)

            # ---- causal scores + softmax ----
            Pbs = []
            rinv1 = stat_p.tile([P, ST], f32, tag="rinv")

            def scores1_qt(qt):
                width = (qt + 1) * P
                pm = ps.tile([P, 512], f32, tag="ps")
                for k in range(DT):
                    nc.tensor.matmul(pm[:, :width],
                                     lhsT=QT[:, k, qt * P:(qt + 1) * P],
                                     rhs=KT[:, k, :width],
                                     start=(k == 0), stop=(k == DT - 1))
                nmax = stat_p.tile([P, 1], f32, tag="nmax")
                nc.vector.reduce_max(nmax, pm[:, :width], axis=X, negate=True)
                diag = dgl_p.tile([P, P], f32, tag="dgl")
                nc.vector.tensor_tensor(out=diag, in0=pm[:, qt * P:width],
                                        in1=trimask, op=ALU.add)
                Pb = pb_p.tile([P, 512], bf16, tag="pb", name=f"pb{qt}")
                rsum = stat_p.tile([P, 1], f32, tag="rsum")
                if qt > 0:
                    rs1 = stat_p.tile([P, 1], f32, tag="rs1")
                    nc.scalar.activation(out=Pb[:, :qt * P], in_=pm[:, :qt * P],
                                         func=ACT_F.Exp, bias=nmax, scale=1.0,
                                         accum_out=rs1)
                    rs2 = stat_p.tile([P, 1], f32, tag="rs2")
                    nc.scalar.activation(out=Pb[:, qt * P:width], in_=diag,
                                         func=ACT_F.Exp, bias=nmax, scale=1.0,
                                         accum_out=rs2)
                    nc.vector.tensor_tensor(out=rsum, in0=rs1, in1=rs2,
                                            op=ALU.add)
                else:
                    nc.scalar.activation(out=Pb[:, :width], in_=diag,
                                         func=ACT_F.Exp, bias=nmax, scale=1.0,
                                         accum_out=rsum)
                nc.vector.reciprocal(out=rinv1[:, qt:qt + 1], in_=rsum)
                Pbs.append(Pb)

            for qt in range(ST):
                scores1_qt(qt)

            # ---- cross-attn K2/V2 emitters (img-side, independent of the
            # tokens; woven into the AV1 loop as PE filler) ----
            # no bk2 — same softmax-shift cancellation as bk1.
            K2T = k2t_p.tile([P, DT, NI_PAD], bf16, tag="k2t")

            def k2t_m(m):
                wk2, mb = (wk2a, m * P) if m < 4 else (wk2b, (m - 4) * P)
                pm = ps.tile([P, 512], f32, tag="ps")
                for k in range(DIT):
                    nc.tensor.matmul(pm[:, :NI],
                                     lhsT=wk2[:, k, mb:mb + P],
                                     rhs=img_sb[:, k, :],
                                     start=(k == 0), stop=(k == DIT - 1))
                nc.scalar.copy(out=K2T[:, m, :NI], in_=pm[:, :NI])

            def v2t_a(a):
                pa = P if a == 0 else NI - P
                for nh in range(2):
                    wv2 = wv2a if nh == 0 else wv2b
                    pm = ps.tile([P, 512], f32, tag="ps")
                    for k in range(DIT):
                        nc.tensor.matmul(
                            pm[:pa, :], lhsT=img_sb[:, k, a * P:a * P + pa],
                            rhs=wv2[:, k, :],
                            start=(k == 0), stop=(k == DIT - 1))
                    nc.scalar.copy(out=V2t[:pa, a, nh * 512:(nh + 1) * 512],
                                   in_=pm[:pa, :])

            def layernorm(xpre, out_sl, gb, bb):
                """xpre [P, D] -> out_sl = norm(xpre) * g + b (g/b optional)."""
                stats = stat_p.tile([P, 2, 6], f32, tag="bnst")
                for sg in range(2):
                    nc.vector.bn_stats(out=stats[:, sg, :],
                                       in_=xpre[:, sg * 512:(sg + 1) * 512])
                mv = stat_p.tile([P, 2], f32, tag="bnmv")
                nc.vector.bn_aggr(out=mv, in_=stats)
                rstd = stat_p.tile([P, 1], f32, tag="rstd")
                nc.scalar.activation(out=rstd, in_=mv[:, 1:2], func=ACT_F.Sqrt,
                                     bias=epst, scale=1.0)
                nc.vector.reciprocal(out=rstd, in_=rstd)
                nmr = stat_p.tile([P, 1], f32, tag="nmr")
                nc.vector.tensor_scalar(out=nmr, in0=mv[:, 0:1], scalar1=rstd,
                                        scalar2=-1.0, op0=ALU.mult,
                                        op1=ALU.mult)
                if gb is None:
                    nc.scalar.activation(out=out_sl, in_=xpre,
                                         func=ACT_F.Identity,
                                         bias=nmr, scale=rstd)
                else:
                    nc.scalar.activation(out=xpre, in_=xpre,
                                         func=ACT_F.Identity,
                                         bias=nmr, scale=rstd)
                    # gain/shift on the otherwise-idle gpsimd (SBUF-only ops)
                    nc.gpsimd.tensor_tensor(out=xpre, in0=xpre, in1=gb,
                                            op=ALU.mult)
                    nc.gpsimd.tensor_tensor(out=out_sl, in0=xpre, in1=bb,
                                            op=ALU.add)

            # ---- AV1 + LN1, with K2/V2 projections woven in as PE filler ----
            PT = pt_p.tile([P, ST, S], bf16, tag="pt")
            x1b = xb_p.tile([P, ST, D], bf16, tag="xb")
            x1T = xt_p.tile([P, ST, DT, P], bf16, tag="x1t", name="x1t")
            x2T = xt_p.tile([P, ST, DT, P], bf16, tag="x2t", name="x2t")
            for qt in range(ST):
                for kt in range(qt + 1):
                    tp = ps.tile([P, 512], bf16, tag="ps", name="tp")
                    nc.tensor.transpose(out=tp[:, :P],
                                        in_=Pbs[qt][:, kt * P:(kt + 1) * P],
                                        identity=ident)
                    nc.scalar.copy(out=PT[:, kt, qt * P:(qt + 1) * P],
                                   in_=tp[:, :P])
                xpre = xpre_p.tile([P, D], bf16, tag="xpre")
                for nh in range(2):
                    pm = ps.tile([P, 512], f32, tag="ps")
                    for kt in range(qt + 1):
                        nc.tensor.matmul(pm, lhsT=PT[:, kt, qt * P:(qt + 1) * P],
                                         rhs=Vt[:, kt, nh * 512:(nh + 1) * 512],
                                         start=(kt == 0), stop=(kt == qt))
                    nc.vector.scalar_tensor_tensor(
                        out=xpre[:, nh * 512:(nh + 1) * 512], in0=pm,
                        scalar=rinv1[:, qt:qt + 1],
                        in1=x0b[:, qt, nh * 512:(nh + 1) * 512],
                        op0=ALU.mult, op1=ALU.add)
                layernorm(xpre, x1b[:, qt, :], g1b, b1b)
                # x1 row transposes ride the (idle) sync queue
                nc.sync.dma_start_transpose(out=x1T[:, qt, :, :],
                                            in_=x1b[:, qt, :])
                if qt == 0:
                    for m in range(4):
                        k2t_m(m)
                elif qt == 1:
                    for m in range(4, DT):
                        k2t_m(m)
                elif qt == 2:
                    v2t_a(0)
                    v2t_a(1)

            # ---- cross attention ----
            Q2T = qk_p.tile([P, DT, S], bf16, tag="qk", name="q2t")

            def q2t_range(s0, s1):
                for m in range(DT):
                    w_sb, mb = (wq2a, m * P) if m < 4 else (wq2b, (m - 4) * P)
                    pm = ps.tile([P, 512], f32, tag="ps", name="pm")
                    for k in range(DT):
                        nc.tensor.matmul(pm[:, :s1 - s0],
                                         lhsT=w_sb[:, k, mb:mb + P],
                                         rhs=x1T[:, s0 // P:s1 // P, k, :],
                                         start=(k == 0), stop=(k == DT - 1))
                    nc.scalar.activation(out=Q2T[:, m, s0:s1],
                                         in_=pm[:, :s1 - s0],
                                         func=ACT_F.Identity,
                                         bias=bqall[:, 2, m:m + 1], scale=1.0)

            P2bs = [None] * ST
            rinv2 = stat_p.tile([P, ST], f32, tag="rinv")

            def scores2_qt(qt):
                pm = ps.tile([P, 512], f32, tag="ps")
                for k in range(DT):
                    nc.tensor.matmul(pm[:, :NI],
                                     lhsT=Q2T[:, k, qt * P:(qt + 1) * P],
                                     rhs=K2T[:, k, :NI],
                                     start=(k == 0), stop=(k == DT - 1))
                nmax = stat_p.tile([P, 1], f32, tag="nmax")
                nc.vector.reduce_max(nmax, pm[:, :NI], axis=X, negate=True)
                P2b = pb_p.tile([P, NI_PAD], bf16, tag="pb", name=f"p2b{qt}")
                nc.gpsimd.memset(P2b[:, NI:], 0.0)
                rsum = stat_p.tile([P, 1], f32, tag="rsum")
                nc.scalar.activation(out=P2b[:, :NI], in_=pm[:, :NI],
                                     func=ACT_F.Exp, bias=nmax, scale=1.0,
                                     accum_out=rsum)
                nc.vector.reciprocal(out=rinv2[:, qt:qt + 1], in_=rsum)
                P2bs[qt] = P2b

            PT2 = pt_p.tile([P, NIT, S], bf16, tag="pt")
            x2b = xb_p.tile([P, ST, D], bf16, tag="xb")

            def av2_qt(qt):
                for kt in range(NIT):
                    tp = ps.tile([P, 512], bf16, tag="ps", name="tp")
                    nc.tensor.transpose(out=tp[:, :P],
                                        in_=P2bs[qt][:, kt * P:(kt + 1) * P],
                                        identity=ident)
                    nc.scalar.copy(out=PT2[:, kt, qt * P:(qt + 1) * P],
                                   in_=tp[:, :P])
                xpre = xpre_p.tile([P, D], bf16, tag="xpre")
                for nh in range(2):
                    pm = ps.tile([P, 512], f32, tag="ps")
                    for kt in range(NIT):
                        nc.tensor.matmul(pm, lhsT=PT2[:, kt, qt * P:(qt + 1) * P],
                                         rhs=V2t[:, kt, nh * 512:(nh + 1) * 512],
                                         start=(kt == 0), stop=(kt == NIT - 1))
                    nc.vector.scalar_tensor_tensor(
                        out=xpre[:, nh * 512:(nh + 1) * 512], in0=pm,
                        scalar=rinv2[:, qt:qt + 1],
                        in1=x1b[:, qt, nh * 512:(nh + 1) * 512],
                        op0=ALU.mult, op1=ALU.add)
                # g2/b2 folded into the vocab weights: x2 = norm(xpre)
                layernorm(xpre, x2b[:, qt, :], None, None)
                nc.sync.dma_start_transpose(out=x2T[:, qt, :, :],
                                            in_=x2b[:, qt, :])

            def vocab_chunk_qt(wp_sb, w, osb_sl, bp_sl, qt):
                pm = ps.tile([P, 512], f32, tag="ps")
                for k in range(DT):
                    nc.tensor.matmul(
                        pm[:, :w], lhsT=x2T[:, qt, k, :],
                        rhs=wp_sb[:, k, :w],
                        start=(k == 0), stop=(k == DT - 1))
                nc.vector.tensor_tensor(out=osb_sl, in0=pm[:, :w],
                                        in1=bp_sl, op=ALU.add)

            # filler outputs (chunk 0 + the 256-wide tail), written per-row
            osb0 = [osb_p.tile([P, CN], bf16, tag="osb0", name=f"osb0_{q}")
                    for q in range(ST)]
            osbt = [osb_p.tile([P, CTAIL], bf16, tag="osbt", name=f"osbt_{q}")
                    for q in range(ST)]

            def filler_qt(qt):
                vocab_chunk_qt(wp_tiles[0], CN, osb0[qt], bp0_bc, qt)
                nc.scalar.dma_start(out=h_out[qt * P:(qt + 1) * P, 0:CN],
                                    in_=osb0[qt])
                vocab_chunk_qt(wpt_sb, CTAIL, osbt[qt], bpt_bc, qt)
                nc.scalar.dma_start(
                    out=h_out[qt * P:(qt + 1) * P, NFULL * CN:V],
                    in_=osbt[qt])

            # ---- cross-attn rows woven with vocab filler ----
            q2t_range(0, P)
            scores2_qt(0)
            q2t_range(P, S)
            av2_qt(0)
            scores2_qt(1)
            filler_qt(0)
            av2_qt(1)
            scores2_qt(2)
            filler_qt(1)
            av2_qt(2)
            scores2_qt(3)
            filler_qt(2)
            av2_qt(3)
            emit_wp(9)
            filler_qt(3)

            # ---- vocab projection, steady-state groups of GRP chunks ----
            ngrp = (NFULL - 1 + GRP - 1) // GRP  # chunks 1..61
            for g in range(ngrp):
                c0 = 1 + g * GRP
                cs = list(range(c0, min(c0 + GRP, NFULL)))
                gw = len(cs) * CN
                off = c0 * CN
                emit_wp(cs[-1] + 1 + 4)
                bp_bc = bp_p.tile([P, GRP * CN], bf16, tag="bp")
                nc.scalar.dma_start(out=bp_bc[:, :gw],
                                    in_=bcast(h_bp, gw, offset=off))
                osb = [osb_p.tile([P, GRP * CN], bf16, tag="osb", bufs=5,
                                  name=f"osb_{g}_{q}") for q in range(ST)]
                for ci, c in enumerate(cs):
                    for qt in range(ST):
                        vocab_chunk_qt(wp_tiles[c], CN,
                                       osb[qt][:, ci * CN:(ci + 1) * CN],
                                       bp_bc[:, ci * CN:(ci + 1) * CN], qt)
                        if ci == len(cs) - 1:
                            out_eng = nc.scalar if qt % 2 == 0 else nc.sync
                            out_eng.dma_start(
                                out=h_out[qt * P:(qt + 1) * P, off:off + gw],
                                in_=osb[qt][:, :gw])

    nc.compile()
    return nc


def _tile_sq(w, kt):
    """[K, N] -> [128, K//128, N] contiguous."""
    k, n = w.shape
    assert k == kt * P
    return np.ascontiguousarray(
        w.reshape(kt, P, n).transpose(1, 0, 2)).astype(BF16)


def _prep_inputs(inputs):
    g = lambda name: np.asarray(inputs[name], dtype=np.float32)
    tokens = np.asarray(inputs["tokens"]).astype(np.int64)
    img = g("img_emb")

    # positional encoding (same closed form as the model definition)
    posn = np.arange(S)[:, None].astype(np.float32)
    i = np.arange(0, D, 2).astype(np.float32)
    ang = posn / np.power(10000.0, i / D)
    pos = np.zeros((S, D), dtype=np.float32)
    pos[:, 0::2] = np.sin(ang)
    pos[:, 1::2] = np.cos(ang)

    # embedding gather + positional add on the host (input prep)
    x0 = g("emb_table")[tokens] + pos[None]          # [B, S, D] fp32
    bv1 = g("bv1")
    bv2 = g("bv2")
    g2 = g("g2")
    b2 = g("b2")

    # bias folds:
    #  - bv1 into the residual stream (A@ (xWv1+bv1) = An@xWv1 + bv1)
    #  - bv2 into LN1's shift b1 (x1' = x1+bv2), compensated in bq2
    #  - g2/b2 (LN2 gain/shift) into the vocab weights/bias
    x0r = (x0 + bv1[None, None, :]).astype(BF16)     # residual stream
    x0 = x0.astype(BF16)                             # projection stream
    b1f = (g("b1") + bv2).astype(BF16)
    bq2f = g("bq2") - bv2 @ g("Wq2")

    wp = g("Wp") * g2[:, None]
    bpf = (b2 @ (g("Wp")) + g("bp")).astype(BF16)
    wp_main = np.ascontiguousarray(
        wp[:, :NFULL * CN].reshape(DT, P, NFULL, CN)
        .transpose(2, 1, 0, 3)).astype(BF16)
    wp_tail = _tile_sq(wp[:, NFULL * CN:], DT)

    def bias_tiled(b):
        return np.ascontiguousarray(b.reshape(DT, P).T).astype(np.float32)

    shared = {
        "wq1": _tile_sq(g("Wq1") * SCALE, DT),
        "wk1": _tile_sq(g("Wk1"), DT),
        "wv1": _tile_sq(g("Wv1"), DT),
        "wq2": _tile_sq(g("Wq2") * SCALE, DT),
        "wk2": _tile_sq(g("Wk2"), DIT),
        "wv2": _tile_sq(g("Wv2"), DIT),
        "wp": wp_main,
        "wpt": wp_tail,
        "bqs": np.ascontiguousarray(np.stack(
            [bias_tiled(g("bq1") * SCALE), bias_tiled(g("bk1")),
             bias_tiled(bq2f * SCALE), bias_tiled(g("bk2"))], axis=1)),
        "bp": bpf,
        "g1": g("g1").astype(BF16), "b1": b1f,
    }
    in_maps = []
    for c in range(N_CORES):
        m = dict(shared)
        m["x0b"] = np.ascontiguousarray(
            x0r[c].reshape(ST, P, D).transpose(1, 0, 2))
        m["x0t"] = np.ascontiguousarray(
            x0[c].T.reshape(DT, P, S).transpose(1, 0, 2))
        m["img_t"] = np.ascontiguousarray(
            img[c].T.reshape(DIT, P, NI).transpose(1, 0, 2)).astype(BF16)
        in_maps.append(m)
    return in_maps


def _ensure_axon_hooks():
    """bass_utils imports antenv.axon_hooks when BASS_TRACE is set; stub it
    if the module is absent so tracing degrades instead of crashing."""
    try:
        import antenv.axon_hooks  # noqa: F401
    except ImportError:
        import types
        mod = types.ModuleType("antenv.axon_hooks")
        mod.get_axon_ntff_profile_hook = lambda: None
        mod.set_axon_ntff_profile_hook = lambda h: None
        sys.modules["antenv.axon_hooks"] = mod


def kernel(**inputs):
    global LAST_RESULTS
    _ensure_axon_hooks()
    from concourse.bass_utils import run_bass_kernel_spmd

    if "nc" not in _CACHE:
        _CACHE["nc"] = _build_program()
    nc = _CACHE["nc"]

    in_maps = _prep_inputs(inputs)
    res = run_bass_kernel_spmd(nc, in_maps, core_ids=list(range(N_CORES)))
    LAST_RESULTS = res
    out = np.stack([res.results[c]["out"].astype(np.float32)
                    for c in range(N_CORES)])
    return out


# revision 41
# speedup vs baseline: 1.0462x; 1.0462x over previous
"""Trainium2 Bass kernel for an 8-batch image-conditioned decoder layer.

Strategy: pure data-parallel over the batch — core c computes batch element c
end-to-end (causal self-attention, cross-attention over the image tokens, both
layernorms, vocab projection).  No collectives.

Schedule v2: the kernel is Tensor-engine bound (~509us of bf16 PE work at
78.6 TF/s), so the layout is organized to keep the PE gapless:
 - the first projection (Q1) runs k-outer across all 8 PSUM banks so it can
   start as soon as the first 384KB k-slab of x/W lands (vs 1.5MB before);
 - host folds biases (bv1 into the residual stream, bv2 into LN1's shift,
   g2/b2 into the vocab weights+bias), removing rank-1 bias matmuls and two
   vector ops per LN2 row;
 - elementwise work is spread (Act: exp/LN-act + Q bias, Vector: reductions +
   K bias + residual, GpSimd: V/P copies) so no chain serializes the PE;
 - one vocab chunk + the 256-wide vocab tail are woven into the
   cross-attention window as PE filler while LN2/x2-transposes drain;
 - the wp stream reuses the attention-weight SBUF ring (slots recycle as the
   attention weights die), and the final vocab group is a single small chunk
   so the last write drains in ~1us.
"""

import os
import sys

for _p in ("/opt/trn_rl_repo", "/root/.axon_site/_ro/trn_rl_repo"):
    if os.path.isdir(_p) and _p not in sys.path:
        sys.path.append(_p)

import numpy as np
import ml_dtypes

BF16 = ml_dtypes.bfloat16

# Problem dims (hardcoded per spec)
V, D, DI, S, B, NI = 32000, 1024, 768, 512, 8, 197
EPS = 1e-5
P = 128
ST = S // P          # 4 seq tiles
DT = D // P          # 8 model-dim tiles
DIT = DI // P        # 6 image-dim tiles
NIT = 2              # image tokens: 197 -> 2 partition tiles (128 + 69)
NI_PAD = 256
CN = 512             # vocab chunk width
NFULL = V // CN      # 62 full chunks
CTAIL = V - NFULL * CN   # 256 tail columns
GRP = 4              # full chunks per steady-state output group
N_CORES = 8
HD = D // 2
SCALE = 1.0 / float(np.sqrt(np.float32(D)))

_CACHE = {}
LAST_RESULTS = None


def _build_program():
    import concourse.bacc as bacc
    import concourse.bass as bass
    import concourse.mybir as mybir
    from concourse.masks import make_identity
    from concourse.tile import TileContext

    f32 = mybir.dt.float32
    bf16 = mybir.dt.bfloat16
    X = mybir.AxisListType.X
    ALU = mybir.AluOpType
    ACT_F = mybir.ActivationFunctionType

    nc = bacc.Bacc("TRN2", target_bir_lowering=False, debug=False,
                   num_devices=N_CORES)

    # ---- I/O ----
    h_x0b = nc.dram_tensor("x0b", [P, ST, D], bf16, kind="ExternalInput")
    h_x0t = nc.dram_tensor("x0t", [P, DT, S], bf16, kind="ExternalInput")
    h_img = nc.dram_tensor("img_t", [P, DIT, NI], bf16, kind="ExternalInput")
    # weight halves are pre-split on the host so each DMA sees 8KB/partition
    # contiguous runs (descriptor efficiency: 1KB rows ~60GB/s, 8KB ~280GB/s)
    h_wq1 = nc.dram_tensor("wq1", [P, DT, D], bf16, kind="ExternalInput")
    h_wk1 = nc.dram_tensor("wk1", [2, P, DT, HD], bf16, kind="ExternalInput")
    h_wv1 = nc.dram_tensor("wv1", [2, P, DT, HD], bf16, kind="ExternalInput")
    h_wq2 = nc.dram_tensor("wq2", [2, P, DT, HD], bf16, kind="ExternalInput")
    h_wk2 = nc.dram_tensor("wk2", [2, P, DIT, HD], bf16, kind="ExternalInput")
    h_wv2 = nc.dram_tensor("wv2", [2, P, DIT, HD], bf16, kind="ExternalInput")
    h_wp = nc.dram_tensor("wp", [NFULL, P, DT, CN], bf16, kind="ExternalInput")
    h_wpt = nc.dram_tensor("wpt", [P, DT, CTAIL], bf16, kind="ExternalInput")
    h_bqs = nc.dram_tensor("bqs", [P, 4, DT], f32, kind="ExternalInput")
    h_bp = nc.dram_tensor("bp", [V], bf16, kind="ExternalInput")
    h_g1 = nc.dram_tensor("g1", [D], bf16, kind="ExternalInput")
    h_b1 = nc.dram_tensor("b1", [D], bf16, kind="ExternalInput")
    h_out = nc.dram_tensor("out", [S, V], bf16, kind="ExternalOutput")

    def bcast(handle, n, offset=0):
        ap = handle[:]
        return bass.AP(tensor=ap.tensor, offset=offset, ap=[[0, P], [1, n]])

    with TileContext(nc) as tc:
        import contextlib
        ctx = contextlib.ExitStack()
        with ctx:
            const = ctx.enter_context(tc.tile_pool(name="const", bufs=1))
            xs_p = ctx.enter_context(tc.tile_pool(name="xs", bufs=1))
            wqs_p = ctx.enter_context(tc.tile_pool(name="wqs", bufs=4))
            wp_p = ctx.enter_context(tc.tile_pool(name="wpp", bufs=4))
            xb_p = ctx.enter_context(tc.tile_pool(name="xb", bufs=2))
            qk_p = ctx.enter_context(tc.tile_pool(name="qk", bufs=2))
            v_p = ctx.enter_context(tc.tile_pool(name="vp", bufs=2))
            k2t_p = ctx.enter_context(tc.tile_pool(name="k2t", bufs=1))
            pb_p = ctx.enter_context(tc.tile_pool(name="pb", bufs=4))
            pt_p = ctx.enter_context(tc.tile_pool(name="pt", bufs=1))
            dgl_p = ctx.enter_context(tc.tile_pool(name="dgl", bufs=1))
            xpre_p = ctx.enter_context(tc.tile_pool(name="xpre", bufs=2))
            stat_p = ctx.enter_context(tc.tile_pool(name="stat", bufs=4))
            wts_p = ctx.enter_context(tc.tile_pool(name="wts", bufs=4))
            xt_p = ctx.enter_context(tc.tile_pool(name="xt", bufs=1))
            bp_p = ctx.enter_context(tc.tile_pool(name="bpp", bufs=1))
            osb_p = ctx.enter_context(tc.tile_pool(name="osb", bufs=4))
            ps = ctx.enter_context(tc.tile_pool(name="ps", bufs=8, space="PSUM"))

            # ---- startup DMA ----
            # DMA throughput scales with per-partition contiguous run length
            # (1KB rows ~60GB/s, 4KB ~180, 8KB ~280), so the first operands
            # ship as 4KB-row halves/quarters split across two queues.
            x0t_lo = xs_p.tile([P, 4, S], bf16, tag="xtl", name="x0tl")
            nc.sync.dma_start(out=x0t_lo, in_=h_x0t[:, 0:4, :])
            wq1q = []
            for qi in range(4):
                t = wqs_p.tile([P, 2, D], bf16, tag="wq1q", name=f"wq1q{qi}")
                nc.scalar.dma_start(out=t, in_=h_wq1[:, 2 * qi:2 * qi + 2, :])
                wq1q.append(t)
            x0t_hi = xs_p.tile([P, 4, S], bf16, tag="xth", name="x0th")
            nc.sync.dma_start(out=x0t_hi, in_=h_x0t[:, 4:DT, :])

            def x0t_of(k):
                return (x0t_lo, k) if k < 4 else (x0t_hi, k - 4)

            # sync: K1 weights, then the first wp chunks (dedicated 4-deep
            # ring so the vocab stream leads independent of attention timing)
            wk1a = wts_p.tile([P, DT, HD], bf16, tag="wts", name="wk1a")
            nc.sync.dma_start(out=wk1a, in_=h_wk1[0])
            wk1b = wts_p.tile([P, DT, HD], bf16, tag="wts", name="wk1b")
            nc.sync.dma_start(out=wk1b, in_=h_wk1[1])
            wp_tiles = {}
            wp_emitted = 0

            def emit_wp(upto):
                nonlocal wp_emitted
                while wp_emitted < min(upto, NFULL):
                    c = wp_emitted
                    t = wp_p.tile([P, DT, CN], bf16, tag="wp", name=f"wp{c}")
                    nc.sync.dma_start(out=t, in_=h_wp[c])
                    wp_tiles[c] = t
                    wp_emitted += 1

            emit_wp(4)

            # scalar: small consts + img + x0b
            bqall = const.tile([P, 4, DT], f32)
            nc.scalar.dma_start(out=bqall, in_=h_bqs[:])
            img_sb = const.tile([P, DIT, NI], bf16)
            nc.scalar.dma_start(out=img_sb, in_=h_img[:])
            g1b = const.tile([P, D], bf16)
            nc.scalar.dma_start(out=g1b, in_=bcast(h_g1, D))
            b1b = const.tile([P, D], bf16)
            nc.scalar.dma_start(out=b1b, in_=bcast(h_b1, D))
            x0b = xb_p.tile([P, ST, D], bf16, tag="xb", name="x0b")
            nc.scalar.dma_start(out=x0b, in_=h_x0b[:])

            # gpsimd: V weights first (vproj needs them at ~30us), then the
            # cross-attn weights; all fire early so the software-DGE has
            # nothing to drain at kernel end.
            wv1a = wts_p.tile([P, DT, HD], bf16, tag="wts", name="wv1a")
            nc.gpsimd.dma_start(out=wv1a, in_=h_wv1[0])
            wv1b = wts_p.tile([P, DT, HD], bf16, tag="wts", name="wv1b")
            nc.gpsimd.dma_start(out=wv1b, in_=h_wv1[1])
            wpt_sb = const.tile([P, DT, CTAIL], bf16)
            nc.gpsimd.dma_start(out=wpt_sb, in_=h_wpt[:])
            bpt_bc = const.tile([P, CTAIL], bf16)
            nc.gpsimd.dma_start(out=bpt_bc, in_=bcast(h_bp, CTAIL,
                                                      offset=NFULL * CN))
            bp0_bc = const.tile([P, CN], bf16)
            nc.gpsimd.dma_start(out=bp0_bc, in_=bcast(h_bp, CN))

            # constants (gpsimd/vector compute, after its early triggers)
            ident = const.tile([P, P], bf16)
            make_identity(nc, ident)
            trimask = const.tile([P, P], f32)
            nc.gpsimd.memset(trimask, 0.0)
            nc.gpsimd.affine_select(
                out=trimask, in_=trimask, compare_op=ALU.is_ge, fill=-1e10,
                base=0, pattern=[[-1, P]], channel_multiplier=1)
            epst = const.tile([P, 1], f32)
            nc.vector.memset(epst, EPS)
            V2t = v_p.tile([P, NIT, D], bf16, tag="v2t", bufs=1)
            nc.vector.memset(V2t[:, 1, :], 0.0)

            # these ride gpsimd after the consts; ring slots free by the time
            # each trigger reaches the head of the queue
            wk2a = wts_p.tile([P, DIT, HD], bf16, tag="wts", name="wk2a")
            nc.gpsimd.dma_start(out=wk2a, in_=h_wk2[0])
            wk2b = wts_p.tile([P, DIT, HD], bf16, tag="wts", name="wk2b")
            nc.gpsimd.dma_start(out=wk2b, in_=h_wk2[1])
            wv2a = wts_p.tile([P, DIT, HD], bf16, tag="wts", name="wv2a")
            nc.gpsimd.dma_start(out=wv2a, in_=h_wv2[0])
            wv2b = wts_p.tile([P, DIT, HD], bf16, tag="wts", name="wv2b")
            nc.gpsimd.dma_start(out=wv2b, in_=h_wv2[1])
            wq2a = wts_p.tile([P, DT, HD], bf16, tag="wts", name="wq2a")
            nc.gpsimd.dma_start(out=wq2a, in_=h_wq2[0])
            wq2b = wts_p.tile([P, DT, HD], bf16, tag="wts", name="wq2b")
            nc.gpsimd.dma_start(out=wq2b, in_=h_wq2[1])

            # ---- Q1 projection, k-outer across all 8 PSUM banks ----
            QT = qk_p.tile([P, DT, S], bf16, tag="qk", name="qt")
            psQ = [ps.tile([P, 512], f32, tag="ps", name=f"psq{m}")
                   for m in range(DT)]
            for k in range(DT):
                xt, kk = x0t_of(k)
                wq, kq = wq1q[k // 2], k % 2
                for m in range(DT):
                    nc.tensor.matmul(psQ[m],
                                     lhsT=wq[:, kq, m * P:(m + 1) * P],
                                     rhs=xt[:, kk, :],
                                     start=(k == 0), stop=(k == DT - 1))
            for m in range(DT):
                nc.scalar.activation(out=QT[:, m, :], in_=psQ[m],
                                     func=ACT_F.Identity,
                                     bias=bqall[:, 0, m:m + 1], scale=1.0)

            # ---- K1 projection, m-outer (x0t fully resident by now) ----
            # note: no bk1 — a bias on K shifts each score row by a constant,
            # which softmax cancels exactly.
            KT = qk_p.tile([P, DT, S], bf16, tag="qk", name="kt")
            for m in range(DT):
                w_sb, mb = (wk1a, m * P) if m < 4 else (wk1b, (m - 4) * P)
                pm = ps.tile([P, 512], f32, tag="ps", name="pmk")
                for k in range(DT):
                    xt, kk = x0t_of(k)
                    nc.tensor.matmul(pm, lhsT=w_sb[:, k, mb:mb + P],
                                     rhs=xt[:, kk, :],
                                     start=(k == 0), stop=(k == DT - 1))
                nc.scalar.copy(out=KT[:, m, :], in_=pm)

            # ---- V projection first: its matmuls cover the Act window that
            # produces the Q/K copies scores depend on ----
            Vt = v_p.tile([P, ST, D], bf16, tag="vt", bufs=1)
            for a in range(ST):
                for nh in range(2):
                    wv = wv1a if nh == 0 else wv1b
                    pm = ps.tile([P, 512], f32, tag="ps")
                    for k in range(DT):
                        xt, kk = x0t_of(k)
                        nc.tensor.matmul(
                            pm, lhsT=xt[:, kk, a * P:(a + 1) * P],
                            rhs=wv[:, k, :],
                            start=(k == 0), stop=(k == DT - 1))
                    nc.vector.tensor_scalar_add(
                        Vt[:, a, nh * 512:(nh + 1) * 512], pm, 0.0)

            # ---- causal scores + softmax ----
            Pbs = []
            rinv1 = stat_p.tile([P, ST], f32, tag="rinv")

            def scores1_qt(qt):
                width = (qt + 1) * P
                pm = ps.tile([P, 512], f32, tag="ps")
                for k in range(DT):
                    nc.tensor.matmul(pm[:, :width],
                                     lhsT=QT[:, k, qt * P:(qt + 1) * P],
                                     rhs=KT[:, k, :width],
                                     start=(k == 0), stop=(k == DT - 1))
                nmax = stat_p.tile([P, 1], f32, tag="nmax")
                nc.vector.reduce_max(nmax, pm[:, :width], axis=X, negate=True)
                diag = dgl_p.tile([P, P], f32, tag="dgl")
                nc.vector.tensor_tensor(out=diag, in0=pm[:, qt * P:width],
                                        in1=trimask, op=ALU.add)
                Pb = pb_p.tile([P, 512], bf16, tag="pb", name=f"pb{qt}")
                rsum = stat_p.tile([P, 1], f32, tag="rsum")
                if qt > 0:
                    rs1 = stat_p.tile([P, 1], f32, tag="rs1")
                    nc.scalar.activation(out=Pb[:, :qt * P], in_=pm[:, :qt * P],
                                         func=ACT_F.Exp, bias=nmax, scale=1.0,
                                         accum_out=rs1)
                    rs2 = stat_p.tile([P, 1], f32, tag="rs2")
                    nc.scalar.activation(out=Pb[:, qt * P:width], in_=diag,
                                         func=ACT_F.Exp, bias=nmax, scale=1.0,
                                         accum_out=rs2)
                    nc.vector.tensor_tensor(out=rsum, in0=rs1, in1=rs2,
                                            op=ALU.add)
                else:
                    nc.scalar.activation(out=Pb[:, :width], in_=diag,
                                         func=ACT_F.Exp, bias=nmax, scale=1.0,
                                         accum_out=rsum)
                nc.vector.reciprocal(out=rinv1[:, qt:qt + 1], in_=rsum)
                Pbs.append(Pb)

            for qt in range(ST):
                scores1_qt(qt)

            # ---- cross-attn K2/V2 emitters (img-side, independent of the
            # tokens; woven into the AV1 loop as PE filler) ----
            # no bk2 — same softmax-shift cancellation as bk1.
            K2T = k2t_p.tile([P, DT, NI_PAD], bf16, tag="k2t")

            def k2t_m(m):
                wk2, mb = (wk2a, m * P) if m < 4 else (wk2b, (m - 4) * P)
                pm = ps.tile([P, 512], f32, tag="ps")
                for k in range(DIT):
                    nc.tensor.matmul(pm[:, :NI],
                                     lhsT=wk2[:, k, mb:mb + P],
                                     rhs=img_sb[:, k, :],
                                     start=(k == 0), stop=(k == DIT - 1))
                nc.scalar.copy(out=K2T[:, m, :NI], in_=pm[:, :NI])

            def v2t_a(a):
                pa = P if a == 0 else NI - P
                for nh in range(2):
                    wv2 = wv2a if nh == 0 else wv2b
                    pm = ps.tile([P, 512], f32, tag="ps")
                    for k in range(DIT):
                        nc.tensor.matmul(
                            pm[:pa, :], lhsT=img_sb[:, k, a * P:a * P + pa],
                            rhs=wv2[:, k, :],
                            start=(k == 0), stop=(k == DIT - 1))
                    nc.scalar.copy(out=V2t[:pa, a, nh * 512:(nh + 1) * 512],
                                   in_=pm[:pa, :])

            def layernorm(xpre, out_sl, gb, bb):
                """xpre [P, D] -> out_sl = norm(xpre) * g + b (g/b optional)."""
                stats = stat_p.tile([P, 2, 6], f32, tag="bnst")
                for sg in range(2):
                    nc.vector.bn_stats(out=stats[:, sg, :],
                                       in_=xpre[:, sg * 512:(sg + 1) * 512])
                mv = stat_p.tile([P, 2], f32, tag="bnmv")
                nc.vector.bn_aggr(out=mv, in_=stats)
                rstd = stat_p.tile([P, 1], f32, tag="rstd")
                nc.scalar.activation(out=rstd, in_=mv[:, 1:2], func=ACT_F.Sqrt,
                                     bias=epst, scale=1.0)
                nc.vector.reciprocal(out=rstd, in_=rstd)
                nmr = stat_p.tile([P, 1], f32, tag="nmr")
                nc.vector.tensor_scalar(out=nmr, in0=mv[:, 0:1], scalar1=rstd,
                                        scalar2=-1.0, op0=ALU.mult,
                                        op1=ALU.mult)
                if gb is None:
                    nc.scalar.activation(out=out_sl, in_=xpre,
                                         func=ACT_F.Identity,
                                         bias=nmr, scale=rstd)
                else:
                    nc.scalar.activation(out=xpre, in_=xpre,
                                         func=ACT_F.Identity,
                                         bias=nmr, scale=rstd)
                    # gain/shift on the otherwise-idle gpsimd (SBUF-only ops)
                    nc.gpsimd.tensor_tensor(out=xpre, in0=xpre, in1=gb,
                                            op=ALU.mult)
                    nc.gpsimd.tensor_tensor(out=out_sl, in0=xpre, in1=bb,
                                            op=ALU.add)

            # ---- AV1 + LN1, with K2/V2 projections woven in as PE filler ----
            PT = pt_p.tile([P, ST, S], bf16, tag="pt")
            x1b = xb_p.tile([P, ST, D], bf16, tag="xb")
            x1T = xt_p.tile([P, ST, DT, P], bf16, tag="x1t", name="x1t")
            x2T = xt_p.tile([P, ST, DT, P], bf16, tag="x2t", name="x2t")
            for qt in range(ST):
                for kt in range(qt + 1):
                    tp = ps.tile([P, 512], bf16, tag="ps", name="tp")
                    nc.tensor.transpose(out=tp[:, :P],
                                        in_=Pbs[qt][:, kt * P:(kt + 1) * P],
                                        identity=ident)
                    nc.scalar.copy(out=PT[:, kt, qt * P:(qt + 1) * P],
                                   in_=tp[:, :P])
                xpre = xpre_p.tile([P, D], bf16, tag="xpre")
                for nh in range(2):
                    pm = ps.tile([P, 512], f32, tag="ps")
                    for kt in range(qt + 1):
                        nc.tensor.matmul(pm, lhsT=PT[:, kt, qt * P:(qt + 1) * P],
                                         rhs=Vt[:, kt, nh * 512:(nh + 1) * 512],
                                         start=(kt == 0), stop=(kt == qt))
                    nc.vector.scalar_tensor_tensor(
                        out=xpre[:, nh * 512:(nh + 1) * 512], in0=pm,
                        scalar=rinv1[:, qt:qt + 1],
                        in1=x0b[:, qt, nh * 512:(nh + 1) * 512],
                        op0=ALU.mult, op1=ALU.add)
                layernorm(xpre, x1b[:, qt, :], g1b, b1b)
                # x1 row transposes ride the (idle) sync queue
                nc.sync.dma_start_transpose(out=x1T[:, qt, :, :],
                                            in_=x1b[:, qt, :])
                if qt == 0:
                    for m in range(4):
                        k2t_m(m)
                elif qt == 1:
                    for m in range(4, DT):
                        k2t_m(m)
                elif qt == 2:
                    v2t_a(0)
                    v2t_a(1)

            # ---- cross attention ----
            Q2T = qk_p.tile([P, DT, S], bf16, tag="qk", name="q2t")

            def q2t_range(s0, s1):
                for m in range(DT):
                    w_sb, mb = (wq2a, m * P) if m < 4 else (wq2b, (m - 4) * P)
                    pm = ps.tile([P, 512], f32, tag="ps", name="pm")
                    for k in range(DT):
                        nc.tensor.matmul(pm[:, :s1 - s0],
                                         lhsT=w_sb[:, k, mb:mb + P],
                                         rhs=x1T[:, s0 // P:s1 // P, k, :],
                                         start=(k == 0), stop=(k == DT - 1))
                    nc.scalar.activation(out=Q2T[:, m, s0:s1],
                                         in_=pm[:, :s1 - s0],
                                         func=ACT_F.Identity,
                                         bias=bqall[:, 2, m:m + 1], scale=1.0)

            P2bs = [None] * ST
            rinv2 = stat_p.tile([P, ST], f32, tag="rinv")

            def scores2_qt(qt):
                pm = ps.tile([P, 512], f32, tag="ps")
                for k in range(DT):
                    nc.tensor.matmul(pm[:, :NI],
                                     lhsT=Q2T[:, k, qt * P:(qt + 1) * P],
                                     rhs=K2T[:, k, :NI],
                                     start=(k == 0), stop=(k == DT - 1))
                nmax = stat_p.tile([P, 1], f32, tag="nmax")
                nc.vector.reduce_max(nmax, pm[:, :NI], axis=X, negate=True)
                P2b = pb_p.tile([P, NI_PAD], bf16, tag="pb", name=f"p2b{qt}")
                nc.gpsimd.memset(P2b[:, NI:], 0.0)
                rsum = stat_p.tile([P, 1], f32, tag="rsum")
                nc.scalar.activation(out=P2b[:, :NI], in_=pm[:, :NI],
                                     func=ACT_F.Exp, bias=nmax, scale=1.0,
                                     accum_out=rsum)
                nc.vector.reciprocal(out=rinv2[:, qt:qt + 1], in_=rsum)
                P2bs[qt] = P2b

            PT2 = pt_p.tile([P, NIT, S], bf16, tag="pt")
            x2b = xb_p.tile([P, ST, D], bf16, tag="xb")

            def av2_qt(qt):
                for kt in range(NIT):
                    tp = ps.tile([P, 512], bf16, tag="ps", name="tp")
                    nc.tensor.transpose(out=tp[:, :P],
                                        in_=P2bs[qt][:, kt * P:(kt + 1) * P],
                                        identity=ident)
                    nc.scalar.copy(out=PT2[:, kt, qt * P:(qt + 1) * P],
                                   in_=tp[:, :P])
                xpre = xpre_p.tile([P, D], bf16, tag="xpre")
                for nh in range(2):
                    pm = ps.tile([P, 512], f32, tag="ps")
                    for kt in range(NIT):
                        nc.tensor.matmul(pm, lhsT=PT2[:, kt, qt * P:(qt + 1) * P],
                                         rhs=V2t[:, kt, nh * 512:(nh + 1) * 512],
                                         start=(kt == 0), stop=(kt == NIT - 1))
                    nc.vector.scalar_tensor_tensor(
                        out=xpre[:, nh * 512:(nh + 1) * 512], in0=pm,
                        scalar=rinv2[:, qt:qt + 1],
                        in1=x1b[:, qt, nh * 512:(nh + 1) * 512],
                        op0=ALU.mult, op1=ALU.add)
                # g2/b2 folded into the vocab weights: x2 = norm(xpre)
                layernorm(xpre, x2b[:, qt, :], None, None)
                nc.sync.dma_start_transpose(out=x2T[:, qt, :, :],
                                            in_=x2b[:, qt, :])

            def vocab_chunk_qt(wp_sb, w, osb_sl, bp_sl, qt):
                pm = ps.tile([P, 512], f32, tag="ps")
                for k in range(DT):
                    nc.tensor.matmul(
                        pm[:, :w], lhsT=x2T[:, qt, k, :],
                        rhs=wp_sb[:, k, :w],
                        start=(k == 0), stop=(k == DT - 1))
                nc.vector.tensor_tensor(out=osb_sl, in0=pm[:, :w],
                                        in1=bp_sl, op=ALU.add)

            # filler outputs (chunk 0 + the 256-wide tail), written per-row
            osb0 = [osb_p.tile([P, CN], bf16, tag="osb0", name=f"osb0_{q}")
                    for q in range(ST)]
            osbt = [osb_p.tile([P, CTAIL], bf16, tag="osbt", name=f"osbt_{q}")
                    for q in range(ST)]

            def filler_qt(qt):
                vocab_chunk_qt(wp_tiles[0], CN, osb0[qt], bp0_bc, qt)
                nc.scalar.dma_start(out=h_out[qt * P:(qt + 1) * P, 0:CN],
                                    in_=osb0[qt])
                vocab_chunk_qt(wpt_sb, CTAIL, osbt[qt], bpt_bc, qt)
                nc.scalar.dma_start(
                    out=h_out[qt * P:(qt + 1) * P, NFULL * CN:V],
                    in_=osbt[qt])

            # ---- cross-attn rows woven with vocab filler ----
            q2t_range(0, P)
            scores2_qt(0)
            q2t_range(P, S)
            av2_qt(0)
            scores2_qt(1)
            filler_qt(0)
            av2_qt(1)
            scores2_qt(2)
            filler_qt(1)
            av2_qt(2)
            scores2_qt(3)
            filler_qt(2)
            av2_qt(3)
            emit_wp(9)
            filler_qt(3)

            # ---- vocab projection, steady-state groups of GRP chunks ----
            ngrp = (NFULL - 1 + GRP - 1) // GRP  # chunks 1..61
            for g in range(ngrp):
                c0 = 1 + g * GRP
                cs = list(range(c0, min(c0 + GRP, NFULL)))
                gw = len(cs) * CN
                off = c0 * CN
                emit_wp(cs[-1] + 1 + 4)
                bp_bc = bp_p.tile([P, GRP * CN], bf16, tag="bp")
                nc.scalar.dma_start(out=bp_bc[:, :gw],
                                    in_=bcast(h_bp, gw, offset=off))
                osb = [osb_p.tile([P, GRP * CN], bf16, tag="osb", bufs=4,
                                  name=f"osb_{g}_{q}") for q in range(ST)]
                for ci, c in enumerate(cs):
                    for qt in range(ST):
                        vocab_chunk_qt(wp_tiles[c], CN,
                                       osb[qt][:, ci * CN:(ci + 1) * CN],
                                       bp_bc[:, ci * CN:(ci + 1) * CN], qt)
                        if ci == len(cs) - 1:
                            out_eng = nc.scalar if qt % 2 == 0 else nc.sync
                            out_eng.dma_start(
                                out=h_out[qt * P:(qt + 1) * P, off:off + gw],
                                in_=osb[qt][:, :gw])

    nc.compile()
    return nc


def _tile_sq(w, kt):
    """[K, N] -> [128, K//128, N] contiguous."""
    k, n = w.shape
    assert k == kt * P
    return np.ascontiguousarray(
        w.reshape(kt, P, n).transpose(1, 0, 2)).astype(BF16)


def _tile_half(w, kt):
    """[K, N] -> [2, 128, K//128, N//2]: column halves, each contiguous
    per partition (8KB descriptor runs)."""
    k, n = w.shape
    assert k == kt * P
    return np.ascontiguousarray(
        w.reshape(kt, P, 2, n // 2).transpose(2, 1, 0, 3)).astype(BF16)


def _prep_inputs(inputs):
    g = lambda name: np.asarray(inputs[name], dtype=np.float32)
    tokens = np.asarray(inputs["tokens"]).astype(np.int64)
    img = g("img_emb")

    # positional encoding (same closed form as the model definition)
    posn = np.arange(S)[:, None].astype(np.float32)
    i = np.arange(0, D, 2).astype(np.float32)
    ang = posn / np.power(10000.0, i / D)
    pos = np.zeros((S, D), dtype=np.float32)
    pos[:, 0::2] = np.sin(ang)
    pos[:, 1::2] = np.cos(ang)

    # embedding gather + positional add on the host (input prep)
    x0 = g("emb_table")[tokens] + pos[None]          # [B, S, D] fp32
    bv1 = g("bv1")
    bv2 = g("bv2")
    g2 = g("g2")
    b2 = g("b2")

    # bias folds:
    #  - bv1 into the residual stream (A@ (xWv1+bv1) = An@xWv1 + bv1)
    #  - bv2 into LN1's shift b1 (x1' = x1+bv2), compensated in bq2
    #  - g2/b2 (LN2 gain/shift) into the vocab weights/bias
    x0r = (x0 + bv1[None, None, :]).astype(BF16)     # residual stream
    x0 = x0.astype(BF16)                             # projection stream
    b1f = (g("b1") + bv2).astype(BF16)
    bq2f = g("bq2") - bv2 @ g("Wq2")

    wp = g("Wp") * g2[:, None]
    bpf = (b2 @ (g("Wp")) + g("bp")).astype(BF16)
    wp_main = np.ascontiguousarray(
        wp[:, :NFULL * CN].reshape(DT, P, NFULL, CN)
        .transpose(2, 1, 0, 3)).astype(BF16)
    wp_tail = _tile_sq(wp[:, NFULL * CN:], DT)

    def bias_tiled(b):
        return np.ascontiguousarray(b.reshape(DT, P).T).astype(np.float32)

    shared = {
        "wq1": _tile_sq(g("Wq1") * SCALE, DT),
        "wk1": _tile_half(g("Wk1"), DT),
        "wv1": _tile_half(g("Wv1"), DT),
        "wq2": _tile_half(g("Wq2") * SCALE, DT),
        "wk2": _tile_half(g("Wk2"), DIT),
        "wv2": _tile_half(g("Wv2"), DIT),
        "wp": wp_main,
        "wpt": wp_tail,
        "bqs": np.ascontiguousarray(np.stack(
            [bias_tiled(g("bq1") * SCALE), bias_tiled(g("bk1")),
             bias_tiled(bq2f * SCALE), bias_tiled(g("bk2"))], axis=1)),
        "bp": bpf,
        "g1": g("g1").astype(BF16), "b1": b1f,
    }
    in_maps = []
    for c in range(N_CORES):
        m = dict(shared)
        m["x0b"] = np.ascontiguousarray(
            x0r[c].reshape(ST, P, D).transpose(1, 0, 2))
        m["x0t"] = np.ascontiguousarray(
            x0[c].T.reshape(DT, P, S).transpose(1, 0, 2))
        m["img_t"] = np.ascontiguousarray(
            img[c].T.reshape(DIT, P, NI).transpose(1, 0, 2)).astype(BF16)
        in_maps.append(m)
    return in_maps


def _ensure_axon_hooks():
    """bass_utils imports antenv.axon_hooks when BASS_TRACE is set; stub it
    if the module is absent so tracing degrades instead of crashing."""
    try:
        import antenv.axon_hooks  # noqa: F401
    except ImportError:
        import types
        mod = types.ModuleType("antenv.axon_hooks")
        mod.get_axon_ntff_profile_hook = lambda: None
        mod.set_axon_ntff_profile_hook = lambda h: None
        sys.modules["antenv.axon_hooks"] = mod


def kernel(**inputs):
    global LAST_RESULTS
    _ensure_axon_hooks()
    from concourse.bass_utils import run_bass_kernel_spmd

    if "nc" not in _CACHE:
        _CACHE["nc"] = _build_program()
    nc = _CACHE["nc"]

    in_maps = _prep_inputs(inputs)
    res = run_bass_kernel_spmd(nc, in_maps, core_ids=list(range(N_CORES)))
    LAST_RESULTS = res
    out = np.stack([res.results[c]["out"].astype(np.float32)
                    for c in range(N_CORES)])
    return out


# revision 53
# speedup vs baseline: 1.0606x; 1.0138x over previous
"""Trainium2 Bass kernel for an 8-batch image-conditioned decoder layer.

Strategy: pure data-parallel over the batch — core c computes batch element c
end-to-end (causal self-attention, cross-attention over the image tokens, both
layernorms, vocab projection).  No collectives.

Schedule v2: the kernel is Tensor-engine bound (~509us of bf16 PE work at
78.6 TF/s), so the layout is organized to keep the PE gapless:
 - the first projection (Q1) runs k-outer across all 8 PSUM banks so it can
   start as soon as the first 384KB k-slab of x/W lands (vs 1.5MB before);
 - host folds biases (bv1 into the residual stream, bv2 into LN1's shift,
   g2/b2 into the vocab weights+bias), removing rank-1 bias matmuls and two
   vector ops per LN2 row;
 - elementwise work is spread (Act: exp/LN-act + Q bias, Vector: reductions +
   K bias + residual, GpSimd: V/P copies) so no chain serializes the PE;
 - one vocab chunk + the 256-wide vocab tail are woven into the
   cross-attention window as PE filler while LN2/x2-transposes drain;
 - the wp stream reuses the attention-weight SBUF ring (slots recycle as the
   attention weights die), and the final vocab group is a single small chunk
   so the last write drains in ~1us.
"""

import os
import sys

for _p in ("/opt/trn_rl_repo", "/root/.axon_site/_ro/trn_rl_repo"):
    if os.path.isdir(_p) and _p not in sys.path:
        sys.path.append(_p)

import numpy as np
import ml_dtypes

BF16 = ml_dtypes.bfloat16

# Problem dims (hardcoded per spec)
V, D, DI, S, B, NI = 32000, 1024, 768, 512, 8, 197
EPS = 1e-5
P = 128
ST = S // P          # 4 seq tiles
DT = D // P          # 8 model-dim tiles
DIT = DI // P        # 6 image-dim tiles
NIT = 2              # image tokens: 197 -> 2 partition tiles (128 + 69)
NI_PAD = 256
CN = 512             # vocab chunk width
NFULL = V // CN      # 62 full chunks
CTAIL = V - NFULL * CN   # 256 tail columns
GRP = 4              # full chunks per steady-state output group
N_CORES = 8
HD = D // 2
SCALE = 1.0 / float(np.sqrt(np.float32(D)))

_CACHE = {}
LAST_RESULTS = None


def _build_program():
    import concourse.bacc as bacc
    import concourse.bass as bass
    import concourse.mybir as mybir
    from concourse.masks import make_identity
    from concourse.tile import TileContext

    f32 = mybir.dt.float32
    bf16 = mybir.dt.bfloat16
    X = mybir.AxisListType.X
    ALU = mybir.AluOpType
    ACT_F = mybir.ActivationFunctionType

    nc = bacc.Bacc("TRN2", target_bir_lowering=False, debug=False,
                   num_devices=N_CORES)

    # ---- I/O ----
    h_x0b = nc.dram_tensor("x0b", [P, ST, D], bf16, kind="ExternalInput")
    h_x0t = nc.dram_tensor("x0t", [P, DT, S], bf16, kind="ExternalInput")
    h_img = nc.dram_tensor("img_t", [P, DIT, NI], bf16, kind="ExternalInput")
    # weight halves are pre-split on the host so each DMA sees 8KB/partition
    # contiguous runs (descriptor efficiency: 1KB rows ~60GB/s, 8KB ~280GB/s)
    h_wq1 = nc.dram_tensor("wq1", [P, DT, D], bf16, kind="ExternalInput")
    h_wk1 = nc.dram_tensor("wk1", [2, P, DT, HD], bf16, kind="ExternalInput")
    h_wv1 = nc.dram_tensor("wv1", [2, P, DT, HD], bf16, kind="ExternalInput")
    h_wq2 = nc.dram_tensor("wq2", [2, P, DT, HD], bf16, kind="ExternalInput")
    h_wk2 = nc.dram_tensor("wk2", [2, P, DIT, HD], bf16, kind="ExternalInput")
    h_wv2 = nc.dram_tensor("wv2", [2, P, DIT, HD], bf16, kind="ExternalInput")
    h_wp = nc.dram_tensor("wp", [NFULL, P, DT, CN], bf16, kind="ExternalInput")
    h_wpt = nc.dram_tensor("wpt", [P, DT, CTAIL], bf16, kind="ExternalInput")
    h_bqs = nc.dram_tensor("bqs", [P, 4, DT], f32, kind="ExternalInput")
    h_bp = nc.dram_tensor("bp", [V], bf16, kind="ExternalInput")
    h_g1 = nc.dram_tensor("g1", [D], bf16, kind="ExternalInput")
    h_b1 = nc.dram_tensor("b1", [D], bf16, kind="ExternalInput")
    h_out = nc.dram_tensor("out", [S, V], bf16, kind="ExternalOutput")

    def bcast(handle, n, offset=0):
        ap = handle[:]
        return bass.AP(tensor=ap.tensor, offset=offset, ap=[[0, P], [1, n]])

    with TileContext(nc) as tc:
        import contextlib
        ctx = contextlib.ExitStack()
        with ctx:
            const = ctx.enter_context(tc.tile_pool(name="const", bufs=1))
            xs_p = ctx.enter_context(tc.tile_pool(name="xs", bufs=1))
            wp_p = ctx.enter_context(tc.tile_pool(name="wpp", bufs=6))
            xb_p = ctx.enter_context(tc.tile_pool(name="xb", bufs=2))
            qk_p = ctx.enter_context(tc.tile_pool(name="qk", bufs=2))
            v_p = ctx.enter_context(tc.tile_pool(name="vp", bufs=2))
            k2t_p = ctx.enter_context(tc.tile_pool(name="k2t", bufs=1))
            pb_p = ctx.enter_context(tc.tile_pool(name="pb", bufs=4))
            pt_p = ctx.enter_context(tc.tile_pool(name="pt", bufs=1))
            dgl_p = ctx.enter_context(tc.tile_pool(name="dgl", bufs=1))
            xpre_p = ctx.enter_context(tc.tile_pool(name="xpre", bufs=2))
            stat_p = ctx.enter_context(tc.tile_pool(name="stat", bufs=4))
            wts_p = ctx.enter_context(tc.tile_pool(name="wts", bufs=4))
            xt_p = ctx.enter_context(tc.tile_pool(name="xt", bufs=1))
            bp_p = ctx.enter_context(tc.tile_pool(name="bpp", bufs=1))
            osb_p = ctx.enter_context(tc.tile_pool(name="osb", bufs=4))
            ps = ctx.enter_context(tc.tile_pool(name="ps", bufs=8, space="PSUM"))

            # ---- startup DMA ----
            # DMA throughput scales with per-partition contiguous run length
            # (1KB rows ~60GB/s, 4KB ~180, 8KB ~280), so the first operands
            # ship as 4KB-row halves/quarters split across two queues.
            x0t_lo = xs_p.tile([P, 4, S], bf16, tag="xtl", name="x0tl")
            nc.sync.dma_start(out=x0t_lo, in_=h_x0t[:, 0:4, :])
            # wq1 halves live in the wp ring (same 8KB/part slot size); they
            # die after the Q projection, handing slots to the early chunks
            wq1h = []
            for qi in range(2):
                t = wp_p.tile([P, 4, D], bf16, tag="wp", name=f"wq1h{qi}")
                nc.scalar.dma_start(out=t, in_=h_wq1[:, 4 * qi:4 * qi + 4, :])
                wq1h.append(t)
            x0t_hi = xs_p.tile([P, 4, S], bf16, tag="xth", name="x0th")
            nc.sync.dma_start(out=x0t_hi, in_=h_x0t[:, 4:DT, :])

            def x0t_of(k):
                return (x0t_lo, k) if k < 4 else (x0t_hi, k - 4)

            # sync: K1 weights, then the first wp chunks (dedicated 4-deep
            # ring so the vocab stream leads independent of attention timing)
            wk1a = wts_p.tile([P, DT, HD], bf16, tag="wts", name="wk1a")
            nc.sync.dma_start(out=wk1a, in_=h_wk1[0])
            wk1b = wts_p.tile([P, DT, HD], bf16, tag="wts", name="wk1b")
            nc.sync.dma_start(out=wk1b, in_=h_wk1[1])
            wp_tiles = {}
            wp_emitted = 0

            def emit_wp(upto):
                nonlocal wp_emitted
                while wp_emitted < min(upto, NFULL):
                    c = wp_emitted
                    t = wp_p.tile([P, DT, CN], bf16, tag="wp", name=f"wp{c}")
                    nc.sync.dma_start(out=t, in_=h_wp[c])
                    wp_tiles[c] = t
                    wp_emitted += 1

            emit_wp(6)

            # scalar: small consts + img + x0b
            bqall = const.tile([P, 4, DT], f32)
            nc.scalar.dma_start(out=bqall, in_=h_bqs[:])
            img_sb = const.tile([P, DIT, NI], bf16)
            nc.scalar.dma_start(out=img_sb, in_=h_img[:])
            g1b = const.tile([P, D], bf16)
            nc.scalar.dma_start(out=g1b, in_=bcast(h_g1, D))
            b1b = const.tile([P, D], bf16)
            nc.scalar.dma_start(out=b1b, in_=bcast(h_b1, D))
            x0b = xb_p.tile([P, ST, D], bf16, tag="xb", name="x0b")
            nc.scalar.dma_start(out=x0b, in_=h_x0b[:])

            # gpsimd: V weights first (vproj needs them at ~30us), then the
            # cross-attn weights; all fire early so the software-DGE has
            # nothing to drain at kernel end.
            wv1a = wts_p.tile([P, DT, HD], bf16, tag="wts", name="wv1a")
            nc.gpsimd.dma_start(out=wv1a, in_=h_wv1[0])
            wv1b = wts_p.tile([P, DT, HD], bf16, tag="wts", name="wv1b")
            nc.gpsimd.dma_start(out=wv1b, in_=h_wv1[1])
            wpt_sb = const.tile([P, DT, CTAIL], bf16)
            nc.gpsimd.dma_start(out=wpt_sb, in_=h_wpt[:])
            bpt_bc = const.tile([P, CTAIL], bf16)
            nc.gpsimd.dma_start(out=bpt_bc, in_=bcast(h_bp, CTAIL,
                                                      offset=NFULL * CN))
            bp0_bc = const.tile([P, CN], bf16)
            nc.gpsimd.dma_start(out=bp0_bc, in_=bcast(h_bp, CN))

            # constants (gpsimd/vector compute, after its early triggers)
            ident = const.tile([P, P], bf16)
            make_identity(nc, ident)
            trimask = const.tile([P, P], f32)
            nc.gpsimd.memset(trimask, 0.0)
            nc.gpsimd.affine_select(
                out=trimask, in_=trimask, compare_op=ALU.is_ge, fill=-1e10,
                base=0, pattern=[[-1, P]], channel_multiplier=1)
            epst = const.tile([P, 1], f32)
            nc.vector.memset(epst, EPS)
            V2t = v_p.tile([P, NIT, D], bf16, tag="v2t", bufs=1)
            nc.vector.memset(V2t[:, 1, :], 0.0)

            # these ride gpsimd after the consts; ring slots free by the time
            # each trigger reaches the head of the queue
            wk2a = wts_p.tile([P, DIT, HD], bf16, tag="wts", name="wk2a")
            nc.gpsimd.dma_start(out=wk2a, in_=h_wk2[0])
            wk2b = wts_p.tile([P, DIT, HD], bf16, tag="wts", name="wk2b")
            nc.gpsimd.dma_start(out=wk2b, in_=h_wk2[1])
            wv2a = wts_p.tile([P, DIT, HD], bf16, tag="wts", name="wv2a")
            nc.gpsimd.dma_start(out=wv2a, in_=h_wv2[0])
            wv2b = wts_p.tile([P, DIT, HD], bf16, tag="wts", name="wv2b")
            nc.gpsimd.dma_start(out=wv2b, in_=h_wv2[1])
            wq2a = wts_p.tile([P, DT, HD], bf16, tag="wts", name="wq2a")
            nc.gpsimd.dma_start(out=wq2a, in_=h_wq2[0])
            wq2b = wts_p.tile([P, DT, HD], bf16, tag="wts", name="wq2b")
            nc.gpsimd.dma_start(out=wq2b, in_=h_wq2[1])

            # ---- Q1 projection, k-outer across all 8 PSUM banks ----
            QT = qk_p.tile([P, DT, S], bf16, tag="qk", name="qt")
            psQ = [ps.tile([P, 512], f32, tag="ps", name=f"psq{m}")
                   for m in range(DT)]
            for k in range(DT):
                xt, kk = x0t_of(k)
                wq, kq = wq1h[k // 4], k % 4
                for m in range(DT):
                    nc.tensor.matmul(psQ[m],
                                     lhsT=wq[:, kq, m * P:(m + 1) * P],
                                     rhs=xt[:, kk, :],
                                     start=(k == 0), stop=(k == DT - 1))
            for m in range(DT):
                nc.scalar.activation(out=QT[:, m, :], in_=psQ[m],
                                     func=ACT_F.Identity,
                                     bias=bqall[:, 0, m:m + 1], scale=1.0)

            # ---- K1 projection, m-outer (x0t fully resident by now) ----
            # note: no bk1 — a bias on K shifts each score row by a constant,
            # which softmax cancels exactly.
            KT = qk_p.tile([P, DT, S], bf16, tag="qk", name="kt")
            for m in range(DT):
                w_sb, mb = (wk1a, m * P) if m < 4 else (wk1b, (m - 4) * P)
                pm = ps.tile([P, 512], f32, tag="ps", name="pmk")
                for k in range(DT):
                    xt, kk = x0t_of(k)
                    nc.tensor.matmul(pm, lhsT=w_sb[:, k, mb:mb + P],
                                     rhs=xt[:, kk, :],
                                     start=(k == 0), stop=(k == DT - 1))
                nc.scalar.copy(out=KT[:, m, :], in_=pm)

            # ---- causal scores + softmax ----
            Pbs = []
            rinv1 = stat_p.tile([P, ST], f32, tag="rinv")

            def scores1_qt(qt):
                width = (qt + 1) * P
                pm = ps.tile([P, 512], f32, tag="ps")
                for k in range(DT):
                    nc.tensor.matmul(pm[:, :width],
                                     lhsT=QT[:, k, qt * P:(qt + 1) * P],
                                     rhs=KT[:, k, :width],
                                     start=(k == 0), stop=(k == DT - 1))
                nmax = stat_p.tile([P, 1], f32, tag="nmax")
                nc.vector.reduce_max(nmax, pm[:, :width], axis=X, negate=True)
                diag = dgl_p.tile([P, P], f32, tag="dgl")
                nc.vector.tensor_tensor(out=diag, in0=pm[:, qt * P:width],
                                        in1=trimask, op=ALU.add)
                Pb = pb_p.tile([P, 512], bf16, tag="pb", name=f"pb{qt}")
                rsum = stat_p.tile([P, 1], f32, tag="rsum")
                if qt > 0:
                    rs1 = stat_p.tile([P, 1], f32, tag="rs1")
                    nc.scalar.activation(out=Pb[:, :qt * P], in_=pm[:, :qt * P],
                                         func=ACT_F.Exp, bias=nmax, scale=1.0,
                                         accum_out=rs1)
                    rs2 = stat_p.tile([P, 1], f32, tag="rs2")
                    nc.scalar.activation(out=Pb[:, qt * P:width], in_=diag,
                                         func=ACT_F.Exp, bias=nmax, scale=1.0,
                                         accum_out=rs2)
                    nc.vector.tensor_tensor(out=rsum, in0=rs1, in1=rs2,
                                            op=ALU.add)
                else:
                    nc.scalar.activation(out=Pb[:, :width], in_=diag,
                                         func=ACT_F.Exp, bias=nmax, scale=1.0,
                                         accum_out=rsum)
                nc.vector.reciprocal(out=rinv1[:, qt:qt + 1], in_=rsum)
                Pbs.append(Pb)

            for qt in range(ST):
                scores1_qt(qt)

            # ---- V projection: fills the PE while softmax chains drain ----
            Vt = v_p.tile([P, ST, D], bf16, tag="vt", bufs=1)
            for a in range(ST):
                for nh in range(2):
                    wv = wv1a if nh == 0 else wv1b
                    pm = ps.tile([P, 512], f32, tag="ps")
                    for k in range(DT):
                        xt, kk = x0t_of(k)
                        nc.tensor.matmul(
                            pm, lhsT=xt[:, kk, a * P:(a + 1) * P],
                            rhs=wv[:, k, :],
                            start=(k == 0), stop=(k == DT - 1))
                    nc.vector.tensor_scalar_add(
                        Vt[:, a, nh * 512:(nh + 1) * 512], pm, 0.0)

            # ---- cross-attn K2/V2 emitters (img-side, independent of the
            # tokens; woven into the AV1 loop as PE filler) ----
            # no bk2 — same softmax-shift cancellation as bk1.
            K2T = k2t_p.tile([P, DT, NI_PAD], bf16, tag="k2t")

            def k2t_m(m):
                wk2, mb = (wk2a, m * P) if m < 4 else (wk2b, (m - 4) * P)
                pm = ps.tile([P, 512], f32, tag="ps")
                for k in range(DIT):
                    nc.tensor.matmul(pm[:, :NI],
                                     lhsT=wk2[:, k, mb:mb + P],
                                     rhs=img_sb[:, k, :],
                                     start=(k == 0), stop=(k == DIT - 1))
                nc.scalar.copy(out=K2T[:, m, :NI], in_=pm[:, :NI])

            def v2t_a(a):
                pa = P if a == 0 else NI - P
                for nh in range(2):
                    wv2 = wv2a if nh == 0 else wv2b
                    pm = ps.tile([P, 512], f32, tag="ps")
                    for k in range(DIT):
                        nc.tensor.matmul(
                            pm[:pa, :], lhsT=img_sb[:, k, a * P:a * P + pa],
                            rhs=wv2[:, k, :],
                            start=(k == 0), stop=(k == DIT - 1))
                    nc.scalar.copy(out=V2t[:pa, a, nh * 512:(nh + 1) * 512],
                                   in_=pm[:pa, :])

            def layernorm(xpre, out_sl, gb, bb):
                """xpre [P, D] -> out_sl = norm(xpre) * g + b (g/b optional)."""
                stats = stat_p.tile([P, 2, 6], f32, tag="bnst")
                for sg in range(2):
                    nc.vector.bn_stats(out=stats[:, sg, :],
                                       in_=xpre[:, sg * 512:(sg + 1) * 512])
                mv = stat_p.tile([P, 2], f32, tag="bnmv")
                nc.vector.bn_aggr(out=mv, in_=stats)
                rstd = stat_p.tile([P, 1], f32, tag="rstd")
                nc.scalar.activation(out=rstd, in_=mv[:, 1:2], func=ACT_F.Sqrt,
                                     bias=epst, scale=1.0)
                nc.vector.reciprocal(out=rstd, in_=rstd)
                nmr = stat_p.tile([P, 1], f32, tag="nmr")
                nc.vector.tensor_scalar(out=nmr, in0=mv[:, 0:1], scalar1=rstd,
                                        scalar2=-1.0, op0=ALU.mult,
                                        op1=ALU.mult)
                if gb is None:
                    nc.scalar.activation(out=out_sl, in_=xpre,
                                         func=ACT_F.Identity,
                                         bias=nmr, scale=rstd)
                else:
                    nc.scalar.activation(out=xpre, in_=xpre,
                                         func=ACT_F.Identity,
                                         bias=nmr, scale=rstd)
                    # gain/shift on the otherwise-idle gpsimd (SBUF-only ops)
                    nc.gpsimd.tensor_tensor(out=xpre, in0=xpre, in1=gb,
                                            op=ALU.mult)
                    nc.gpsimd.tensor_tensor(out=out_sl, in0=xpre, in1=bb,
                                            op=ALU.add)

            # ---- AV1 + LN1, with K2/V2 projections woven in as PE filler ----
            PT = pt_p.tile([P, ST, S], bf16, tag="pt")
            x1b = xb_p.tile([P, ST, D], bf16, tag="xb")
            # x1T/x2T split by row so a consumer of row r doesn't inherit a
            # semaphore threshold covering later rows' transposes
            x1T0 = xt_p.tile([P, 1, DT, P], bf16, tag="x1t0", name="x1t0")
            x1TB = xt_p.tile([P, 3, DT, P], bf16, tag="x1tb", name="x1tb")
            x2Tr = [xt_p.tile([P, DT, P], bf16, tag=f"x2t{q}", name=f"x2t{q}")
                    for q in range(ST)]
            for qt in range(ST):
                for kt in range(qt + 1):
                    tp = ps.tile([P, 512], bf16, tag="ps", name="tp")
                    nc.tensor.transpose(out=tp[:, :P],
                                        in_=Pbs[qt][:, kt * P:(kt + 1) * P],
                                        identity=ident)
                    nc.scalar.copy(out=PT[:, kt, qt * P:(qt + 1) * P],
                                   in_=tp[:, :P])
                xpre = xpre_p.tile([P, D], bf16, tag="xpre")
                for nh in range(2):
                    pm = ps.tile([P, 512], f32, tag="ps")
                    for kt in range(qt + 1):
                        nc.tensor.matmul(pm, lhsT=PT[:, kt, qt * P:(qt + 1) * P],
                                         rhs=Vt[:, kt, nh * 512:(nh + 1) * 512],
                                         start=(kt == 0), stop=(kt == qt))
                    nc.vector.scalar_tensor_tensor(
                        out=xpre[:, nh * 512:(nh + 1) * 512], in0=pm,
                        scalar=rinv1[:, qt:qt + 1],
                        in1=x0b[:, qt, nh * 512:(nh + 1) * 512],
                        op0=ALU.mult, op1=ALU.add)
                layernorm(xpre, x1b[:, qt, :], g1b, b1b)
                # x1 row transposes ride the (idle) sync queue
                if qt == 0:
                    nc.sync.dma_start_transpose(out=x1T0[:, 0, :, :],
                                                in_=x1b[:, 0, :])
                else:
                    nc.sync.dma_start_transpose(out=x1TB[:, qt - 1, :, :],
                                                in_=x1b[:, qt, :])
                if qt == 0:
                    for m in range(4):
                        k2t_m(m)
                elif qt == 1:
                    for m in range(4, DT):
                        k2t_m(m)
                elif qt == 2:
                    v2t_a(0)
                    v2t_a(1)

            # ---- cross attention ----
            Q2T = qk_p.tile([P, DT, S], bf16, tag="qk", name="q2t")

            def q2t_range(s0, s1):
                src = x1T0 if s0 == 0 else x1TB
                o = 0 if s0 == 0 else (s0 // P) - 1
                for m in range(DT):
                    w_sb, mb = (wq2a, m * P) if m < 4 else (wq2b, (m - 4) * P)
                    pm = ps.tile([P, 512], f32, tag="ps", name="pm")
                    for k in range(DT):
                        nc.tensor.matmul(pm[:, :s1 - s0],
                                         lhsT=w_sb[:, k, mb:mb + P],
                                         rhs=src[:, o:o + (s1 - s0) // P, k, :],
                                         start=(k == 0), stop=(k == DT - 1))
                    nc.scalar.activation(out=Q2T[:, m, s0:s1],
                                         in_=pm[:, :s1 - s0],
                                         func=ACT_F.Identity,
                                         bias=bqall[:, 2, m:m + 1], scale=1.0)

            P2bs = [None] * ST
            rinv2 = stat_p.tile([P, ST], f32, tag="rinv")

            def scores2_qt(qt):
                pm = ps.tile([P, 512], f32, tag="ps")
                for k in range(DT):
                    nc.tensor.matmul(pm[:, :NI],
                                     lhsT=Q2T[:, k, qt * P:(qt + 1) * P],
                                     rhs=K2T[:, k, :NI],
                                     start=(k == 0), stop=(k == DT - 1))
                nmax = stat_p.tile([P, 1], f32, tag="nmax")
                nc.vector.reduce_max(nmax, pm[:, :NI], axis=X, negate=True)
                P2b = pb_p.tile([P, NI_PAD], bf16, tag="pb", name=f"p2b{qt}")
                nc.gpsimd.memset(P2b[:, NI:], 0.0)
                rsum = stat_p.tile([P, 1], f32, tag="rsum")
                nc.scalar.activation(out=P2b[:, :NI], in_=pm[:, :NI],
                                     func=ACT_F.Exp, bias=nmax, scale=1.0,
                                     accum_out=rsum)
                nc.vector.reciprocal(out=rinv2[:, qt:qt + 1], in_=rsum)
                P2bs[qt] = P2b

            PT2 = pt_p.tile([P, NIT, S], bf16, tag="pt")
            x2b = xb_p.tile([P, ST, D], bf16, tag="xb")

            def av2_qt(qt):
                for kt in range(NIT):
                    tp = ps.tile([P, 512], bf16, tag="ps", name="tp")
                    nc.tensor.transpose(out=tp[:, :P],
                                        in_=P2bs[qt][:, kt * P:(kt + 1) * P],
                                        identity=ident)
                    nc.scalar.copy(out=PT2[:, kt, qt * P:(qt + 1) * P],
                                   in_=tp[:, :P])
                xpre = xpre_p.tile([P, D], bf16, tag="xpre")
                for nh in range(2):
                    pm = ps.tile([P, 512], f32, tag="ps")
                    for kt in range(NIT):
                        nc.tensor.matmul(pm, lhsT=PT2[:, kt, qt * P:(qt + 1) * P],
                                         rhs=V2t[:, kt, nh * 512:(nh + 1) * 512],
                                         start=(kt == 0), stop=(kt == NIT - 1))
                    nc.vector.scalar_tensor_tensor(
                        out=xpre[:, nh * 512:(nh + 1) * 512], in0=pm,
                        scalar=rinv2[:, qt:qt + 1],
                        in1=x1b[:, qt, nh * 512:(nh + 1) * 512],
                        op0=ALU.mult, op1=ALU.add)
                # g2/b2 folded into the vocab weights: x2 = norm(xpre)
                layernorm(xpre, x2b[:, qt, :], None, None)
                nc.sync.dma_start_transpose(out=x2Tr[qt][:, :, :],
                                            in_=x2b[:, qt, :])

            def vocab_chunk_qt(wp_sb, w, osb_sl, bp_sl, qt):
                pm = ps.tile([P, 512], f32, tag="ps")
                for k in range(DT):
                    nc.tensor.matmul(
                        pm[:, :w], lhsT=x2Tr[qt][:, k, :],
                        rhs=wp_sb[:, k, :w],
                        start=(k == 0), stop=(k == DT - 1))
                nc.vector.tensor_tensor(out=osb_sl, in0=pm[:, :w],
                                        in1=bp_sl, op=ALU.add)

            # filler outputs (chunk 0 + the 256-wide tail), written per-row
            osb0 = [osb_p.tile([P, CN], bf16, tag="osb0", name=f"osb0_{q}")
                    for q in range(ST)]
            osbt = [osb_p.tile([P, CTAIL], bf16, tag="osbt", name=f"osbt_{q}")
                    for q in range(ST)]

            def filler_qt(qt):
                vocab_chunk_qt(wp_tiles[0], CN, osb0[qt], bp0_bc, qt)
                nc.scalar.dma_start(out=h_out[qt * P:(qt + 1) * P, 0:CN],
                                    in_=osb0[qt])
                vocab_chunk_qt(wpt_sb, CTAIL, osbt[qt], bpt_bc, qt)
                nc.scalar.dma_start(
                    out=h_out[qt * P:(qt + 1) * P, NFULL * CN:V],
                    in_=osbt[qt])

            # ---- cross-attn rows woven with vocab filler ----
            q2t_range(0, P)
            scores2_qt(0)
            q2t_range(P, S)
            av2_qt(0)
            scores2_qt(1)
            filler_qt(0)
            av2_qt(1)
            scores2_qt(2)
            filler_qt(1)
            av2_qt(2)
            scores2_qt(3)
            filler_qt(2)
            av2_qt(3)
            filler_qt(3)
            emit_wp(9)

            # ---- vocab projection, steady-state groups of GRP chunks ----
            ngrp = (NFULL - 1 + GRP - 1) // GRP  # chunks 1..61
            for g in range(ngrp):
                c0 = 1 + g * GRP
                cs = list(range(c0, min(c0 + GRP, NFULL)))
                gw = len(cs) * CN
                off = c0 * CN
                emit_wp(cs[-1] + 1 + 4)
                bp_bc = bp_p.tile([P, GRP * CN], bf16, tag="bp")
                nc.scalar.dma_start(out=bp_bc[:, :gw],
                                    in_=bcast(h_bp, gw, offset=off))
                osb = [osb_p.tile([P, GRP * CN], bf16, tag="osb", bufs=4,
                                  name=f"osb_{g}_{q}") for q in range(ST)]
                for ci, c in enumerate(cs):
                    for qt in range(ST):
                        vocab_chunk_qt(wp_tiles[c], CN,
                                       osb[qt][:, ci * CN:(ci + 1) * CN],
                                       bp_bc[:, ci * CN:(ci + 1) * CN], qt)
                        if ci == len(cs) - 1:
                            out_eng = nc.scalar if qt % 2 == 0 else nc.sync
                            out_eng.dma_start(
                                out=h_out[qt * P:(qt + 1) * P, off:off + gw],
                                in_=osb[qt][:, :gw])

    nc.compile()
    return nc


def _tile_sq(w, kt):
    """[K, N] -> [128, K//128, N] contiguous."""
    k, n = w.shape
    assert k == kt * P
    return np.ascontiguousarray(
        w.reshape(kt, P, n).transpose(1, 0, 2)).astype(BF16)


def _tile_half(w, kt):
    """[K, N] -> [2, 128, K//128, N//2]: column halves, each contiguous
    per partition (8KB descriptor runs)."""
    k, n = w.shape
    assert k == kt * P
    return np.ascontiguousarray(
        w.reshape(kt, P, 2, n // 2).transpose(2, 1, 0, 3)).astype(BF16)


def _prep_inputs(inputs):
    g = lambda name: np.asarray(inputs[name], dtype=np.float32)
    tokens = np.asarray(inputs["tokens"]).astype(np.int64)
    img = g("img_emb")

    # positional encoding (same closed form as the model definition)
    posn = np.arange(S)[:, None].astype(np.float32)
    i = np.arange(0, D, 2).astype(np.float32)
    ang = posn / np.power(10000.0, i / D)
    pos = np.zeros((S, D), dtype=np.float32)
    pos[:, 0::2] = np.sin(ang)
    pos[:, 1::2] = np.cos(ang)

    # embedding gather + positional add on the host (input prep)
    x0 = g("emb_table")[tokens] + pos[None]          # [B, S, D] fp32
    bv1 = g("bv1")
    bv2 = g("bv2")
    g2 = g("g2")
    b2 = g("b2")

    # bias folds:
    #  - bv1 into the residual stream (A@ (xWv1+bv1) = An@xWv1 + bv1)
    #  - bv2 into LN1's shift b1 (x1' = x1+bv2), compensated in bq2
    #  - g2/b2 (LN2 gain/shift) into the vocab weights/bias
    x0r = (x0 + bv1[None, None, :]).astype(BF16)     # residual stream
    x0 = x0.astype(BF16)                             # projection stream
    b1f = (g("b1") + bv2).astype(BF16)
    bq2f = g("bq2") - bv2 @ g("Wq2")

    wp = g("Wp") * g2[:, None]
    bpf = (b2 @ (g("Wp")) + g("bp")).astype(BF16)
    wp_main = np.ascontiguousarray(
        wp[:, :NFULL * CN].reshape(DT, P, NFULL, CN)
        .transpose(2, 1, 0, 3)).astype(BF16)
    wp_tail = _tile_sq(wp[:, NFULL * CN:], DT)

    def bias_tiled(b):
        return np.ascontiguousarray(b.reshape(DT, P).T).astype(np.float32)

    shared = {
        "wq1": _tile_sq(g("Wq1") * SCALE, DT),
        "wk1": _tile_half(g("Wk1"), DT),
        "wv1": _tile_half(g("Wv1"), DT),
        "wq2": _tile_half(g("Wq2") * SCALE, DT),
        "wk2": _tile_half(g("Wk2"), DIT),
        "wv2": _tile_half(g("Wv2"), DIT),
        "wp": wp_main,
        "wpt": wp_tail,
        "bqs": np.ascontiguousarray(np.stack(
            [bias_tiled(g("bq1") * SCALE), bias_tiled(g("bk1")),
             bias_tiled(bq2f * SCALE), bias_tiled(g("bk2"))], axis=1)),
        "bp": bpf,
        "g1": g("g1").astype(BF16), "b1": b1f,
    }
    in_maps = []
    for c in range(N_CORES):
        m = dict(shared)
        m["x0b"] = np.ascontiguousarray(
            x0r[c].reshape(ST, P, D).transpose(1, 0, 2))
        m["x0t"] = np.ascontiguousarray(
            x0[c].T.reshape(DT, P, S).transpose(1, 0, 2))
        m["img_t"] = np.ascontiguousarray(
            img[c].T.reshape(DIT, P, NI).transpose(1, 0, 2)).astype(BF16)
        in_maps.append(m)
    return in_maps


def _ensure_axon_hooks():
    """bass_utils imports antenv.axon_hooks when BASS_TRACE is set; stub it
    if the module is absent so tracing degrades instead of crashing."""
    try:
        import antenv.axon_hooks  # noqa: F401
    except ImportError:
        import types
        mod = types.ModuleType("antenv.axon_hooks")
        mod.get_axon_ntff_profile_hook = lambda: None
        mod.set_axon_ntff_profile_hook = lambda h: None
        sys.modules["antenv.axon_hooks"] = mod


def kernel(**inputs):
    global LAST_RESULTS
    _ensure_axon_hooks()
    from concourse.bass_utils import run_bass_kernel_spmd

    if "nc" not in _CACHE:
        _CACHE["nc"] = _build_program()
    nc = _CACHE["nc"]

    in_maps = _prep_inputs(inputs)
    res = run_bass_kernel_spmd(nc, in_maps, core_ids=list(range(N_CORES)))
    LAST_RESULTS = res
    out = np.stack([res.results[c]["out"].astype(np.float32)
                    for c in range(N_CORES)])
    return out
